# revision 11
# baseline (speedup 1.0000x reference)
"""Clements-mesh kernel for Trainium2 (8 NeuronCores, data-parallel).

The reference applies 64 layers of 2x2 Givens-like rotations (alternating
even/odd pair offsets) to x [32768, 256].  Each layer is right-multiplication
by a 256x256 block-diagonal orthogonal matrix U_l, so the whole network is
out = x @ (U_0 @ U_1 @ ... @ U_63) = x @ M with M a dense 256x256 matrix that
only depends on the tiny theta [64, 128].  M is built on host in float64;
the device kernel is a single [4096, 256] @ [256, 256] matmul per core,
which is memory-bound (4 MiB in + 4 MiB out per core).

Precision: the PE runs bf16 at 1 cycle/row but fp32 at 4 (and fp32r requires
explicitly rounded TF32-like inputs), so the matmul is done as a 3-term
bf16 split: x = xh + xl, M = Mh + Ml (bf16 each, RTNE), and
out ~= xh@Mh + xh@Ml + xl@Mh accumulated exactly in fp32 PSUM.  The dropped
xl@Ml term is ~2^-18 relative; measured end-to-end rel err vs the reference
is ~4.5e-6 (the reference itself deviates ~2.2e-6 from float64).

Device layout: TensorE contracts over the partition dim of both operands, so
x is shipped feature-major (host pre-transpose), split hi/lo on host:
  xin [4, 128, 4096] bf16   (xh_kc0, xh_kc1, xl_kc0, xl_kc1; kc = feature
                             chunk of 128, free dim = batch)
  min [4, 128,  256] bf16   (Mh_kc0, Mh_kc1, Ml_kc0, Ml_kc1)
  outT[2, 128, 4096] f32    (feature chunk jc, feature-in-chunk, batch)
out^T[j, b] = sum_k M[k, j] * x^T[k, b]; PSUM banks are drained to SBUF by
DVE/ACT (DMA cannot read PSUM) and DMAed out feature-major; the host
transposes back while gathering.
"""

import sys

import numpy as np

if "/opt/trn_rl_repo" not in sys.path:
    sys.path.insert(0, "/opt/trn_rl_repo")

import concourse.bass as bass
import concourse.mybir as mybir
from concourse.tile import TileContext

D = 256          # feature dim
B = 32768        # batch
NCORES = 8
BS = B // NCORES  # 4096 batch rows per core
P = 128          # SBUF partitions
NB = 512         # batch columns per matmul (one fp32 PSUM bank)
NBLK = BS // NB  # 8
F32 = mybir.dt.float32
BF16 = mybir.dt.bfloat16

# (x term, M term) pairs accumulated per PSUM bank: hh + hl + lh.
# x terms: 0=xh_kc0, 1=xh_kc1, 2=xl_kc0, 3=xl_kc1; M terms likewise.
TERMS = [(0, 0), (1, 1), (0, 2), (1, 3), (2, 0), (3, 1)]

_NC_CACHE = {}


def _fused_matrix(theta: np.ndarray) -> np.ndarray:
    """M = U_0 @ U_1 @ ... @ U_63 in float64."""
    theta = np.asarray(theta, dtype=np.float64)
    M = np.eye(D, dtype=np.float64)
    for layer in range(theta.shape[0]):
        th = theta[layer]
        if layer % 2 == 0:
            npairs = D // 2
            i_idx = np.arange(0, D - 1, 2)
        else:
            npairs = D // 2 - 1
            i_idx = np.arange(1, D - 2, 2)
        j_idx = i_idx + 1
        c = np.cos(2.0 * th[:npairs])
        s = np.sin(2.0 * th[:npairs])
        Mi = M[:, i_idx].copy()
        Mj = M[:, j_idx]
        M[:, i_idx] = c * Mi + s * Mj
        M[:, j_idx] = s * Mi - c * Mj
    return M


def _split_bf16(a32: np.ndarray):
    """a32 (f32) -> (hi, lo) bf16 with hi + lo ~= a32 (RTNE both)."""
    import ml_dtypes

    hi = a32.astype(ml_dtypes.bfloat16)
    lo = (a32 - hi.astype(np.float32)).astype(ml_dtypes.bfloat16)
    return hi, lo


def _legalize_waits(nc: bass.Bass, max_waits: int = 1) -> None:
    """Split instructions carrying more than ``max_waits`` sync waits.

    This walrus build rejects instructions with multiple sync-wait commands
    (e.g. the Tile tail drain waits on every engine/DMA-lane sem at once).
    Excess waits move to injected same-engine NoOps immediately before the
    instruction, which is semantically identical: the engine blocks on each
    wait in sequence before executing the original instruction.
    """
    for fn in nc.m.functions:
        for blk in fn.blocks:
            insts = blk.instructions
            i = 0
            while i < len(insts):
                inst = insts[i]
                si = inst.sync_info
                if si is not None and len(si.on_wait) > max_waits:
                    waits = list(si.on_wait)
                    keep, extra = waits[-max_waits:], waits[:-max_waits]
                    for k, w in enumerate(extra):
                        nop = mybir.InstNoOp(
                            name=f"{inst.name}-waitsplit-{k}", ins=[], outs=[]
                        )
                        nop.engine = inst.engine
                        nop.sync_info = mybir.SyncInfo(on_wait=[w], on_update=[])
                        insts.insert(i, nop)
                        i += 1
                    inst.sync_info = mybir.SyncInfo(
                        on_wait=keep, on_update=list(si.on_update)
                    )
                i += 1


def _build_nc() -> bass.Bass:
    nc = bass.Bass()
    xin = nc.declare_dram_parameter("xin", [4, P, BS], BF16, isOutput=False)
    min_ = nc.declare_dram_parameter("min", [4, P, D], BF16, isOutput=False)
    outT = nc.declare_dram_parameter("outT", [2, P, BS], F32, isOutput=True)

    with TileContext(nc) as tc:
        with (
            tc.tile_pool(name="mw", bufs=1) as mpool,
            tc.tile_pool(name="xin_p", bufs=1) as xpool,
            tc.tile_pool(name="oo", bufs=1) as opool,
            tc.tile_pool(name="ps", bufs=8, space="PSUM") as pspool,
        ):
            # Single coalesced DMA per input: only 8 HWDGE lanes exist and a
            # lane-reuse wait + data wait would exceed the 1-sync-wait-per-DMA
            # codegen limit, so keep the total DMA instruction count <= 8.
            m_sb = mpool.tile([P, 4 * D], BF16)
            nc.sync.dma_start(
                out=m_sb[:].rearrange("p (t d) -> p t d", t=4),
                in_=min_.transpose([1, 0, 2]),
            )

            x_all = xpool.tile([P, 4 * BS], BF16, name="x_all")
            nc.sync.dma_start(
                out=x_all[:].rearrange("p (t b) -> p t b", t=4),
                in_=xin.transpose([1, 0, 2]),
            )
            x_sb = [x_all[:, t * BS : (t + 1) * BS] for t in range(4)]

            o_sb = [
                opool.tile([P, BS], F32, tag=f"o{jc}", name=f"o_sb{jc}")
                for jc in range(2)
            ]

            for bb in range(NBLK):
                bsl = slice(bb * NB, (bb + 1) * NB)
                for jc in range(2):
                    ps = pspool.tile([P, NB], F32, tag="ps", name=f"ps_{bb}_{jc}")
                    for i, (x_t, m_t) in enumerate(TERMS):
                        nc.tensor.matmul(
                            ps[:],
                            lhsT=m_sb[:, m_t * D + jc * P : m_t * D + (jc + 1) * P],
                            rhs=x_sb[x_t][:, bsl],
                            start=(i == 0),
                            stop=(i == len(TERMS) - 1),
                        )
                    if jc == 0:
                        nc.vector.tensor_copy(o_sb[jc][:, bsl], ps[:])
                    else:
                        nc.scalar.copy(o_sb[jc][:, bsl], ps[:])

            # One out-DMA per jc: copies for a jc all run on one engine, so
            # the data dependency is a single engine-sem wait, and the total
            # DMA count (4) keeps the tail drain under the 8-sync-wait cap.
            for jc in range(2):
                nc.sync.dma_start(out=outT[jc], in_=o_sb[jc][:])

    _legalize_waits(nc)
    return nc


def _get_nc() -> bass.Bass:
    if "nc" not in _NC_CACHE:
        _NC_CACHE["nc"] = _build_nc()
    return _NC_CACHE["nc"]


def _make_in_maps(x: np.ndarray, theta: np.ndarray):
    x = np.ascontiguousarray(np.asarray(x), dtype=np.float32)
    M32 = _fused_matrix(theta).astype(np.float32)
    mh, ml = _split_bf16(M32)
    m_arr = np.stack(
        [mh[:P], mh[P:], ml[:P], ml[P:]], axis=0
    )  # [4, 128, 256] bf16
    m_arr = np.ascontiguousarray(m_arr)

    xr = x.reshape(NCORES, BS, D)
    in_maps = []
    for c in range(NCORES):
        shard_t = np.ascontiguousarray(xr[c].T)  # [256, 4096] f32
        xh, xl = _split_bf16(shard_t)
        xin = np.stack([xh[:P], xh[P:], xl[:P], xl[P:]], axis=0)
        in_maps.append({"xin": np.ascontiguousarray(xin), "min": m_arr})
    return in_maps


def _gather(results) -> np.ndarray:
    out = np.empty((B, D), dtype=np.float32)
    for c in range(NCORES):
        outT = results[c]["outT"].reshape(D, BS)
        out[c * BS : (c + 1) * BS] = outT.T
    return out


def run(x: np.ndarray, theta: np.ndarray, trace: bool = False):
    """Returns (out, BassKernelResults)."""
    from concourse.bass_utils import run_bass_kernel_spmd

    in_maps = _make_in_maps(x, theta)
    res = run_bass_kernel_spmd(
        _get_nc(), in_maps, list(range(NCORES)), trace=trace
    )
    return _gather(res.results), res


def kernel(x: np.ndarray, theta: np.ndarray) -> np.ndarray:
    out, _ = run(x, theta, trace=False)
    return out


# revision 12
# speedup vs baseline: 1.3022x; 1.3022x over previous
"""Clements-mesh kernel for Trainium2 (8 NeuronCores, data-parallel).

The reference applies 64 layers of 2x2 Givens-like rotations (alternating
even/odd pair offsets) to x [32768, 256].  Each layer is right-multiplication
by a 256x256 block-diagonal orthogonal matrix U_l, so the whole network is
out = x @ (U_0 @ U_1 @ ... @ U_63) = x @ M with M a dense 256x256 matrix that
only depends on the tiny theta [64, 128].  M is built on host in float64;
the device kernel is a single [4096, 256] @ [256, 256] matmul per core,
which is memory-bound (4 MiB in + 4 MiB out per core).

Precision: the PE runs bf16 at 1 cycle/row but fp32 at 4 (and fp32r requires
explicitly rounded TF32-like inputs), so the matmul is done as a 3-term
bf16 split: x = xh + xl, M = Mh + Ml (bf16 each, RTNE), and
out ~= xh@Mh + xh@Ml + xl@Mh accumulated exactly in fp32 PSUM.  The dropped
xl@Ml term is ~2^-18 relative; measured end-to-end rel err vs the reference
is ~4.5e-6 (the reference itself deviates ~2.2e-6 from float64).

Device layout: TensorE contracts over the partition dim of both operands, so
x is shipped feature-major (host pre-transpose), split hi/lo on host:
  xin [4, 128, 4096] bf16   (xh_kc0, xh_kc1, xl_kc0, xl_kc1; kc = feature
                             chunk of 128, free dim = batch)
  min [4, 128,  256] bf16   (Mh_kc0, Mh_kc1, Ml_kc0, Ml_kc1)
  outT[2, 128, 4096] f32    (feature chunk jc, feature-in-chunk, batch)
out^T[j, b] = sum_k M[k, j] * x^T[k, b]; PSUM banks are drained to SBUF by
DVE/ACT (DMA cannot read PSUM) and DMAed out feature-major; the host
transposes back while gathering.
"""

import sys

import numpy as np

if "/opt/trn_rl_repo" not in sys.path:
    sys.path.insert(0, "/opt/trn_rl_repo")

import concourse.bass as bass
import concourse.mybir as mybir
from concourse.tile import TileContext

D = 256          # feature dim
B = 32768        # batch
NCORES = 8
BS = B // NCORES  # 4096 batch rows per core
P = 128          # SBUF partitions
NB = 512         # batch columns per matmul (one fp32 PSUM bank)
NBLK = BS // NB  # 8
F32 = mybir.dt.float32
BF16 = mybir.dt.bfloat16

# (x term, M term) pairs accumulated per PSUM bank: hh + hl + lh.
# x terms: 0=xh_kc0, 1=xh_kc1, 2=xl_kc0, 3=xl_kc1; M terms likewise.
TERMS = [(0, 0), (1, 1), (0, 2), (1, 3), (2, 0), (3, 1)]

_NC_CACHE = {}


def _fused_matrix(theta: np.ndarray) -> np.ndarray:
    """M = U_0 @ U_1 @ ... @ U_63 in float64."""
    theta = np.asarray(theta, dtype=np.float64)
    M = np.eye(D, dtype=np.float64)
    for layer in range(theta.shape[0]):
        th = theta[layer]
        if layer % 2 == 0:
            npairs = D // 2
            i_idx = np.arange(0, D - 1, 2)
        else:
            npairs = D // 2 - 1
            i_idx = np.arange(1, D - 2, 2)
        j_idx = i_idx + 1
        c = np.cos(2.0 * th[:npairs])
        s = np.sin(2.0 * th[:npairs])
        Mi = M[:, i_idx].copy()
        Mj = M[:, j_idx]
        M[:, i_idx] = c * Mi + s * Mj
        M[:, j_idx] = s * Mi - c * Mj
    return M


def _split_bf16(a32: np.ndarray):
    """a32 (f32) -> (hi, lo) bf16 with hi + lo ~= a32 (RTNE both)."""
    import ml_dtypes

    hi = a32.astype(ml_dtypes.bfloat16)
    lo = (a32 - hi.astype(np.float32)).astype(ml_dtypes.bfloat16)
    return hi, lo


def _legalize_waits(nc: bass.Bass, max_waits: int = 1) -> None:
    """Split instructions carrying more than ``max_waits`` sync waits.

    This walrus build rejects instructions with multiple sync-wait commands
    (e.g. the Tile tail drain waits on every engine/DMA-lane sem at once).
    Excess waits move to injected same-engine NoOps immediately before the
    instruction, which is semantically identical: the engine blocks on each
    wait in sequence before executing the original instruction.
    """
    for fn in nc.m.functions:
        for blk in fn.blocks:
            insts = blk.instructions
            i = 0
            while i < len(insts):
                inst = insts[i]
                si = inst.sync_info
                if si is not None and len(si.on_wait) > max_waits:
                    waits = list(si.on_wait)
                    keep, extra = waits[-max_waits:], waits[:-max_waits]
                    for k, w in enumerate(extra):
                        nop = mybir.InstNoOp(
                            name=f"{inst.name}-waitsplit-{k}", ins=[], outs=[]
                        )
                        nop.engine = inst.engine
                        nop.sync_info = mybir.SyncInfo(on_wait=[w], on_update=[])
                        insts.insert(i, nop)
                        i += 1
                    inst.sync_info = mybir.SyncInfo(
                        on_wait=keep, on_update=list(si.on_update)
                    )
                i += 1


def _build_nc() -> bass.Bass:
    nc = bass.Bass()
    xin = nc.declare_dram_parameter("xin", [4, P, BS], BF16, isOutput=False)
    min_ = nc.declare_dram_parameter("min", [4, P, D], BF16, isOutput=False)
    outT = nc.declare_dram_parameter("outT", [2, P, BS], F32, isOutput=True)

    with TileContext(nc) as tc:
        with (
            tc.tile_pool(name="mw", bufs=1) as mpool,
            tc.tile_pool(name="xin_p", bufs=1) as xpool,
            tc.tile_pool(name="oo", bufs=1) as opool,
            tc.tile_pool(name="ps", bufs=8, space="PSUM") as pspool,
        ):
            # Single coalesced DMA per input: only 8 HWDGE lanes exist and a
            # lane-reuse wait + data wait would exceed the 1-sync-wait-per-DMA
            # codegen limit, so keep the total DMA instruction count <= 8.
            m_sb = mpool.tile([P, 4 * D], BF16)
            nc.sync.dma_start(
                out=m_sb[:].rearrange("p (t d) -> p t d", t=4),
                in_=min_.transpose([1, 0, 2]),
            )

            x_all = xpool.tile([P, 4 * BS], BF16, name="x_all")
            nc.sync.dma_start(
                out=x_all[:].rearrange("p (t b) -> p t b", t=4),
                in_=xin.transpose([1, 0, 2]),
            )
            x_sb = [x_all[:, t * BS : (t + 1) * BS] for t in range(4)]

            o_sb = [
                opool.tile([P, BS], F32, tag=f"o{jc}", name=f"o_sb{jc}")
                for jc in range(2)
            ]

            for bb in range(NBLK):
                bsl = slice(bb * NB, (bb + 1) * NB)
                for jc in range(2):
                    ps = pspool.tile([P, NB], F32, tag="ps", name=f"ps_{bb}_{jc}")
                    for i, (x_t, m_t) in enumerate(TERMS):
                        nc.tensor.matmul(
                            ps[:],
                            lhsT=m_sb[:, m_t * D + jc * P : m_t * D + (jc + 1) * P],
                            rhs=x_sb[x_t][:, bsl],
                            start=(i == 0),
                            stop=(i == len(TERMS) - 1),
                        )
                    if jc == 0:
                        nc.vector.tensor_copy(o_sb[jc][:, bsl], ps[:])
                    else:
                        nc.scalar.copy(o_sb[jc][:, bsl], ps[:])

            # One out-DMA per jc: copies for a jc all run on one engine, so
            # the data dependency is a single engine-sem wait, and the total
            # DMA count (4) keeps the tail drain under the 8-sync-wait cap.
            for jc in range(2):
                nc.sync.dma_start(out=outT[jc], in_=o_sb[jc][:])

    _legalize_waits(nc)
    return nc


def _build_nc_raw() -> bass.Bass:
    """Hand-scheduled version: chunked DMA/PE/copy/DMA-out pipeline with
    explicit semaphores, no Tile tail barrier (saves ~25 us vs Tile)."""
    from contextlib import ExitStack

    nc = bass.Bass()
    xin = nc.declare_dram_parameter("xin", [4, P, BS], BF16, isOutput=False)
    min_ = nc.declare_dram_parameter("min", [4, P, D], BF16, isOutput=False)
    outT = nc.declare_dram_parameter("outT", [2, P, BS], F32, isOutput=True)

    CH = 4              # batch chunks for the in-DMA pipeline
    CB = BS // CH       # 1024 batch columns per chunk
    NWARM = 8           # HAM warmup matmuls while the first chunk streams in

    with ExitStack() as ctx:
        m_sb = ctx.enter_context(nc.sbuf_tensor("m_sb", [P, 4 * D], BF16))
        x_sb = ctx.enter_context(nc.sbuf_tensor("x_sb", [P, 4 * BS], BF16))
        o_sb = ctx.enter_context(nc.sbuf_tensor("o_sb", [P, 2 * BS], F32))
        ps = [
            ctx.enter_context(nc.psum_tensor(f"ps{b}", [P, NB], F32))
            for b in range(8)
        ]
        in_sem = ctx.enter_context(nc.semaphore("in_sem"))
        pe_sem = ctx.enter_context(nc.semaphore("pe_sem"))
        dve_sem = ctx.enter_context(nc.semaphore("dve_sem"))
        act_sem = ctx.enter_context(nc.semaphore("act_sem"))
        out_sem = ctx.enter_context(nc.semaphore("out_sem"))
        block = ctx.enter_context(nc.Block())

        # Group g = 2*bb + jc fills PSUM bank g % 8 with 6 accumulated
        # matmuls; jc0 banks drain on DVE, jc1 banks on ACT.

        @block.sync
        def _(sp):
            sp.dma_start(
                out=m_sb[:].rearrange("p (t d) -> p t d", t=4),
                in_=min_.transpose([1, 0, 2]),
            ).then_inc(in_sem, 16)
            for c in range(CH):
                for t in range(4):
                    sp.dma_start(
                        out=x_sb[:, t * BS + c * CB : t * BS + (c + 1) * CB],
                        in_=xin[t][:, c * CB : (c + 1) * CB],
                    ).then_inc(in_sem, 16)
            for c in range(CH):
                for jc in range(2):
                    sem = dve_sem if jc == 0 else act_sem
                    sp.wait_ge(sem, 2 * (c + 1))
                    sp.dma_start(
                        out=outT[jc][:, c * CB : (c + 1) * CB],
                        in_=o_sb[:, jc * BS + c * CB : jc * BS + (c + 1) * CB],
                    ).then_inc(out_sem, 16)
            sp.wait_ge(out_sem, 16 * 2 * CH)

        @block.tensor
        def _(pe):
            # Warm the PE HAM clock gate on garbage SBUF while chunk 0 lands;
            # bank 7's real group later overwrites this via start=True.
            for _w in range(NWARM):
                pe.matmul(
                    ps[7][:],
                    lhsT=m_sb[:, 0:P],
                    rhs=x_sb[:, 0:NB],
                    start=True,
                    stop=True,
                )
            g = 0
            for c in range(CH):
                pe.wait_ge(in_sem, 16 * (1 + 4 * (c + 1)))
                for bb in range(c * (CB // NB), (c + 1) * (CB // NB)):
                    for jc in range(2):
                        bank = g % 8
                        if g >= 8:
                            prev = g - 8
                            sem = dve_sem if prev % 2 == 0 else act_sem
                            pe.wait_ge(sem, prev // 2 + 1)
                        mm = None
                        for i, (x_t, m_t) in enumerate(TERMS):
                            mm = pe.matmul(
                                ps[bank][:],
                                lhsT=m_sb[
                                    :, m_t * D + jc * P : m_t * D + (jc + 1) * P
                                ],
                                rhs=x_sb[
                                    :, x_t * BS + bb * NB : x_t * BS + (bb + 1) * NB
                                ],
                                start=(i == 0),
                                stop=(i == len(TERMS) - 1),
                            )
                        mm.then_inc(pe_sem, 1)
                        g += 1

        @block.vector
        def _(dve):
            for i in range(NBLK):  # jc0 groups: g = 2i
                dve.wait_ge(pe_sem, 2 * i + 1)
                dve.tensor_copy(
                    o_sb[:, i * NB : (i + 1) * NB], ps[(2 * i) % 8][:]
                ).then_inc(dve_sem, 1)

        @block.scalar
        def _(act):
            for i in range(NBLK):  # jc1 groups: g = 2i + 1
                act.wait_ge(pe_sem, 2 * i + 2)
                act.copy(
                    o_sb[:, BS + i * NB : BS + (i + 1) * NB], ps[(2 * i + 1) % 8][:]
                ).then_inc(act_sem, 1)

    _legalize_waits(nc)
    return nc


RAW = True


def _get_nc() -> bass.Bass:
    if "nc" not in _NC_CACHE:
        _NC_CACHE["nc"] = _build_nc_raw() if RAW else _build_nc()
    return _NC_CACHE["nc"]


def _make_in_maps(x: np.ndarray, theta: np.ndarray):
    x = np.ascontiguousarray(np.asarray(x), dtype=np.float32)
    M32 = _fused_matrix(theta).astype(np.float32)
    mh, ml = _split_bf16(M32)
    m_arr = np.stack(
        [mh[:P], mh[P:], ml[:P], ml[P:]], axis=0
    )  # [4, 128, 256] bf16
    m_arr = np.ascontiguousarray(m_arr)

    xr = x.reshape(NCORES, BS, D)
    in_maps = []
    for c in range(NCORES):
        shard_t = np.ascontiguousarray(xr[c].T)  # [256, 4096] f32
        xh, xl = _split_bf16(shard_t)
        xin = np.stack([xh[:P], xh[P:], xl[:P], xl[P:]], axis=0)
        in_maps.append({"xin": np.ascontiguousarray(xin), "min": m_arr})
    return in_maps


def _gather(results) -> np.ndarray:
    out = np.empty((B, D), dtype=np.float32)
    for c in range(NCORES):
        outT = results[c]["outT"].reshape(D, BS)
        out[c * BS : (c + 1) * BS] = outT.T
    return out


def run(x: np.ndarray, theta: np.ndarray, trace: bool = False):
    """Returns (out, BassKernelResults)."""
    from concourse.bass_utils import run_bass_kernel_spmd

    in_maps = _make_in_maps(x, theta)
    res = run_bass_kernel_spmd(
        _get_nc(), in_maps, list(range(NCORES)), trace=trace
    )
    return _gather(res.results), res


def kernel(x: np.ndarray, theta: np.ndarray) -> np.ndarray:
    out, _ = run(x, theta, trace=False)
    return out


# revision 18
# speedup vs baseline: 1.3826x; 1.0617x over previous
"""Clements-mesh kernel for Trainium2 (8 NeuronCores, data-parallel).

The reference applies 64 layers of 2x2 Givens-like rotations (alternating
even/odd pair offsets) to x [32768, 256].  Each layer is right-multiplication
by a 256x256 block-diagonal orthogonal matrix U_l, so the whole network is
out = x @ (U_0 @ U_1 @ ... @ U_63) = x @ M with M a dense 256x256 matrix that
only depends on the tiny theta [64, 128].  M is built on host in float64;
the device kernel is a single [4096, 256] @ [256, 256] matmul per core,
which is memory-bound (4 MiB in + 4 MiB out per core).

Precision: the PE runs bf16 at 1 cycle/row but fp32 at 4 (and fp32r requires
explicitly rounded TF32-like inputs), so the matmul is done as a 3-term
bf16 split: x = xh + xl, M = Mh + Ml (bf16 each, RTNE), and
out ~= xh@Mh + xh@Ml + xl@Mh accumulated exactly in fp32 PSUM.  The dropped
xl@Ml term is ~2^-18 relative; measured end-to-end rel err vs the reference
is ~4.5e-6 (the reference itself deviates ~2.2e-6 from float64).

Device layout: TensorE contracts over the partition dim of both operands, so
x is shipped feature-major (host pre-transpose), split hi/lo on host:
  xin [4, 128, 4096] bf16   (xh_kc0, xh_kc1, xl_kc0, xl_kc1; kc = feature
                             chunk of 128, free dim = batch)
  min [4, 128,  256] bf16   (Mh_kc0, Mh_kc1, Ml_kc0, Ml_kc1)
  outT[2, 128, 4096] f32    (feature chunk jc, feature-in-chunk, batch)
out^T[j, b] = sum_k M[k, j] * x^T[k, b]; PSUM banks are drained to SBUF by
DVE/ACT (DMA cannot read PSUM) and DMAed out feature-major; the host
transposes back while gathering.
"""

import sys

import numpy as np

if "/opt/trn_rl_repo" not in sys.path:
    sys.path.insert(0, "/opt/trn_rl_repo")

import concourse.bass as bass
import concourse.mybir as mybir
from concourse.tile import TileContext

D = 256          # feature dim
B = 32768        # batch
NCORES = 8
BS = B // NCORES  # 4096 batch rows per core
P = 128          # SBUF partitions
NB = 512         # batch columns per matmul (one fp32 PSUM bank)
NBLK = BS // NB  # 8
F32 = mybir.dt.float32
BF16 = mybir.dt.bfloat16

# (x term, M term) pairs accumulated per PSUM bank: hh + hl + lh.
# x terms: 0=xh_kc0, 1=xh_kc1, 2=xl_kc0, 3=xl_kc1; M terms likewise.
TERMS = [(0, 0), (1, 1), (0, 2), (1, 3), (2, 0), (3, 1)]

_NC_CACHE = {}


def _fused_matrix(theta: np.ndarray) -> np.ndarray:
    """M = U_0 @ U_1 @ ... @ U_63 in float64."""
    theta = np.asarray(theta, dtype=np.float64)
    M = np.eye(D, dtype=np.float64)
    for layer in range(theta.shape[0]):
        th = theta[layer]
        if layer % 2 == 0:
            npairs = D // 2
            i_idx = np.arange(0, D - 1, 2)
        else:
            npairs = D // 2 - 1
            i_idx = np.arange(1, D - 2, 2)
        j_idx = i_idx + 1
        c = np.cos(2.0 * th[:npairs])
        s = np.sin(2.0 * th[:npairs])
        Mi = M[:, i_idx].copy()
        Mj = M[:, j_idx]
        M[:, i_idx] = c * Mi + s * Mj
        M[:, j_idx] = s * Mi - c * Mj
    return M


def _split_bf16(a32: np.ndarray):
    """a32 (f32) -> (hi, lo) bf16 with hi + lo ~= a32 (RTNE both)."""
    import ml_dtypes

    hi = a32.astype(ml_dtypes.bfloat16)
    lo = (a32 - hi.astype(np.float32)).astype(ml_dtypes.bfloat16)
    return hi, lo


def _legalize_waits(nc: bass.Bass, max_waits: int = 1) -> None:
    """Split instructions carrying more than ``max_waits`` sync waits.

    This walrus build rejects instructions with multiple sync-wait commands
    (e.g. the Tile tail drain waits on every engine/DMA-lane sem at once).
    Excess waits move to injected same-engine NoOps immediately before the
    instruction, which is semantically identical: the engine blocks on each
    wait in sequence before executing the original instruction.
    """
    for fn in nc.m.functions:
        for blk in fn.blocks:
            insts = blk.instructions
            i = 0
            while i < len(insts):
                inst = insts[i]
                si = inst.sync_info
                if si is not None and len(si.on_wait) > max_waits:
                    waits = list(si.on_wait)
                    keep, extra = waits[-max_waits:], waits[:-max_waits]
                    for k, w in enumerate(extra):
                        nop = mybir.InstNoOp(
                            name=f"{inst.name}-waitsplit-{k}", ins=[], outs=[]
                        )
                        nop.engine = inst.engine
                        nop.sync_info = mybir.SyncInfo(on_wait=[w], on_update=[])
                        insts.insert(i, nop)
                        i += 1
                    inst.sync_info = mybir.SyncInfo(
                        on_wait=keep, on_update=list(si.on_update)
                    )
                i += 1


def _build_nc() -> bass.Bass:
    nc = bass.Bass()
    xin = nc.declare_dram_parameter("xin", [4, P, BS], BF16, isOutput=False)
    min_ = nc.declare_dram_parameter("min", [4, P, D], BF16, isOutput=False)
    outT = nc.declare_dram_parameter("outT", [2, P, BS], F32, isOutput=True)

    with TileContext(nc) as tc:
        with (
            tc.tile_pool(name="mw", bufs=1) as mpool,
            tc.tile_pool(name="xin_p", bufs=1) as xpool,
            tc.tile_pool(name="oo", bufs=1) as opool,
            tc.tile_pool(name="ps", bufs=8, space="PSUM") as pspool,
        ):
            # Single coalesced DMA per input: only 8 HWDGE lanes exist and a
            # lane-reuse wait + data wait would exceed the 1-sync-wait-per-DMA
            # codegen limit, so keep the total DMA instruction count <= 8.
            m_sb = mpool.tile([P, 4 * D], BF16)
            nc.sync.dma_start(
                out=m_sb[:].rearrange("p (t d) -> p t d", t=4),
                in_=min_.transpose([1, 0, 2]),
            )

            x_all = xpool.tile([P, 4 * BS], BF16, name="x_all")
            nc.sync.dma_start(
                out=x_all[:].rearrange("p (t b) -> p t b", t=4),
                in_=xin.transpose([1, 0, 2]),
            )
            x_sb = [x_all[:, t * BS : (t + 1) * BS] for t in range(4)]

            o_sb = [
                opool.tile([P, BS], F32, tag=f"o{jc}", name=f"o_sb{jc}")
                for jc in range(2)
            ]

            for bb in range(NBLK):
                bsl = slice(bb * NB, (bb + 1) * NB)
                for jc in range(2):
                    ps = pspool.tile([P, NB], F32, tag="ps", name=f"ps_{bb}_{jc}")
                    for i, (x_t, m_t) in enumerate(TERMS):
                        nc.tensor.matmul(
                            ps[:],
                            lhsT=m_sb[:, m_t * D + jc * P : m_t * D + (jc + 1) * P],
                            rhs=x_sb[x_t][:, bsl],
                            start=(i == 0),
                            stop=(i == len(TERMS) - 1),
                        )
                    if jc == 0:
                        nc.vector.tensor_copy(o_sb[jc][:, bsl], ps[:])
                    else:
                        nc.scalar.copy(o_sb[jc][:, bsl], ps[:])

            # One out-DMA per jc: copies for a jc all run on one engine, so
            # the data dependency is a single engine-sem wait, and the total
            # DMA count (4) keeps the tail drain under the 8-sync-wait cap.
            for jc in range(2):
                nc.sync.dma_start(out=outT[jc], in_=o_sb[jc][:])

    _legalize_waits(nc)
    return nc


def _strip_barriers(nc: bass.Bass) -> None:
    """Remove the exit all-engine EVSEM butterfly + drains (~4-7 us).

    The exit barrier only synchronizes engine stream ends; our semaphore
    protocol (SP waits for every out-DMA receipt, GpSimd then resets the
    semaphores) already guarantees completion ordering.  The *init* barrier
    is kept: it orders the GpSimd start-of-run semaphore clears before any
    engine's first wait, making the NEFF robust to dirty device semaphore
    state left by a crashed or foreign predecessor kernel.
    """
    fn = nc.m.functions[0]

    def is_barrier(inst):
        tn = type(inst).__name__
        if tn == "InstDrain":
            return True
        return tn == "InstEventSemaphore" and inst.name.startswith("barrier")

    blk = fn.blocks[-1]
    insts = blk.instructions
    keep = [i for i in insts if not is_barrier(i)]
    if len(keep) != len(insts):
        insts[:] = keep


def _build_nc_raw() -> bass.Bass:
    """Hand-scheduled version: chunked DMA/PE/copy/DMA-out pipeline with
    explicit semaphores, no Tile tail barrier (saves ~25 us vs Tile)."""
    from contextlib import ExitStack

    nc = bass.Bass()
    xin = nc.declare_dram_parameter("xin", [4, P, BS], BF16, isOutput=False)
    min_ = nc.declare_dram_parameter("min", [4, P, D], BF16, isOutput=False)
    outT = nc.declare_dram_parameter("outT", [2, P, BS], F32, isOutput=True)

    # Graded batch chunks: small first chunk so the PE starts early, larger
    # later chunks for DMA efficiency (PE consumes ~2x slower than DMA).
    CHUNKS = [512, 512, 1024, 1024, 1024]
    assert sum(CHUNKS) == BS
    NWARM = 3           # HAM warmup matmuls while the first chunk streams in

    with ExitStack() as ctx:
        m_sb = ctx.enter_context(nc.sbuf_tensor("m_sb", [P, 4 * D], BF16))
        x_sb = ctx.enter_context(nc.sbuf_tensor("x_sb", [P, 4 * BS], BF16))
        o_sb = ctx.enter_context(nc.sbuf_tensor("o_sb", [P, 2 * BS], F32))
        ps = [
            ctx.enter_context(nc.psum_tensor(f"ps{b}", [P, NB], F32))
            for b in range(8)
        ]
        in_sem = ctx.enter_context(nc.semaphore("in_sem"))
        pe_sem = ctx.enter_context(nc.semaphore("pe_sem"))
        dve_sem = ctx.enter_context(nc.semaphore("dve_sem"))
        act_sem = ctx.enter_context(nc.semaphore("act_sem"))
        out_sem = ctx.enter_context(nc.semaphore("out_sem"))
        block = ctx.enter_context(nc.Block())

        # Group g = 2*bb + jc fills PSUM bank g % 8 with 6 accumulated
        # matmuls; jc0 banks drain on DVE, jc1 banks on ACT.

        @block.sync
        def _(sp):
            sp.dma_start(
                out=m_sb[:].rearrange("p (t d) -> p t d", t=4),
                in_=min_.transpose([1, 0, 2]),
            ).then_inc(in_sem, 16)
            off = 0
            for cb in CHUNKS:
                for t in range(4):
                    sp.dma_start(
                        out=x_sb[:, t * BS + off : t * BS + off + cb],
                        in_=xin[t][:, off : off + cb],
                    ).then_inc(in_sem, 16)
                off += cb
            # One out-DMA per PSUM bank drain (256 KB), issued in group
            # completion order: earliest possible start, small exposed tail.
            for bb in range(NBLK):
                for jc in range(2):
                    sem = dve_sem if jc == 0 else act_sem
                    sp.wait_ge(sem, bb + 1)
                    sp.dma_start(
                        out=outT[jc][:, bb * NB : (bb + 1) * NB],
                        in_=o_sb[:, jc * BS + bb * NB : jc * BS + (bb + 1) * NB],
                    ).then_inc(out_sem, 16)
            sp.wait_ge(out_sem, 16 * 2 * NBLK)

        @block.tensor
        def _(pe):
            # Warm the PE HAM clock gate on garbage SBUF while chunk 0 lands;
            # bank 7's real group later overwrites this via start=True.
            for _w in range(NWARM):
                pe.matmul(
                    ps[7][:],
                    lhsT=m_sb[:, 0:P],
                    rhs=x_sb[:, 0:NB],
                    start=True,
                    stop=True,
                )
            g = 0
            ndma = 1
            off = 0
            for cb in CHUNKS:
                ndma += 4
                pe.wait_ge(in_sem, 16 * ndma)
                for bb in range(off // NB, (off + cb) // NB):
                    for jc in range(2):
                        bank = g % 8
                        if g >= 8:
                            prev = g - 8
                            sem = dve_sem if prev % 2 == 0 else act_sem
                            pe.wait_ge(sem, prev // 2 + 1)
                        mm = None
                        for i, (x_t, m_t) in enumerate(TERMS):
                            mm = pe.matmul(
                                ps[bank][:],
                                lhsT=m_sb[
                                    :, m_t * D + jc * P : m_t * D + (jc + 1) * P
                                ],
                                rhs=x_sb[
                                    :, x_t * BS + bb * NB : x_t * BS + (bb + 1) * NB
                                ],
                                start=(i == 0),
                                stop=(i == len(TERMS) - 1),
                            )
                        mm.then_inc(pe_sem, 1)
                        g += 1
                off += cb

        @block.vector
        def _(dve):
            # Delay ops: give GpSimd's start-of-run semaphore clears time to
            # land before our first wait could observe stale values.
            dve.memset(o_sb[:, 0:NB], 0.0)
            dve.memset(o_sb[:, 0:NB], 0.0)
            for i in range(NBLK):  # jc0 groups: g = 2i
                dve.wait_ge(pe_sem, 2 * i + 1)
                dve.tensor_copy(
                    o_sb[:, i * NB : (i + 1) * NB], ps[(2 * i) % 8][:]
                ).then_inc(dve_sem, 1)

        @block.scalar
        def _(act):
            # Delay ops, same reason as the DVE memsets.
            act.copy(o_sb[:, BS : BS + NB], o_sb[:, BS : BS + NB])
            act.copy(o_sb[:, BS : BS + NB], o_sb[:, BS : BS + NB])
            for i in range(NBLK):  # jc1 groups: g = 2i + 1
                act.wait_ge(pe_sem, 2 * i + 2)
                act.copy(
                    o_sb[:, BS + i * NB : BS + (i + 1) * NB], ps[(2 * i + 1) % 8][:]
                ).then_inc(act_sem, 1)

        @block.gpsimd
        def _(gp):
            # Start-of-run: zero our semaphores so a dirty predecessor
            # kernel (crashed run, foreign NEFF) cannot poison the waits.
            # Consumers' first waits are all >= ~1 us into their streams
            # (warmups / delay ops / DMA transfer latency), far after these.
            for s in (in_sem, pe_sem, dve_sem, act_sem, out_sem):
                gp.sem_clear(s)
            # End-of-run: wait for the last output-DMA write receipt, then
            # reset semaphores so the loaded NEFF is re-executable.
            gp.wait_ge(out_sem, 16 * 2 * NBLK)
            for s in (in_sem, pe_sem, dve_sem, act_sem, out_sem):
                gp.sem_clear(s)

    _strip_barriers(nc)
    _legalize_waits(nc)
    return nc


RAW = True


def _get_nc() -> bass.Bass:
    if "nc" not in _NC_CACHE:
        _NC_CACHE["nc"] = _build_nc_raw() if RAW else _build_nc()
    return _NC_CACHE["nc"]


def _make_in_maps(x: np.ndarray, theta: np.ndarray):
    x = np.ascontiguousarray(np.asarray(x), dtype=np.float32)
    M32 = _fused_matrix(theta).astype(np.float32)
    mh, ml = _split_bf16(M32)
    m_arr = np.stack(
        [mh[:P], mh[P:], ml[:P], ml[P:]], axis=0
    )  # [4, 128, 256] bf16
    m_arr = np.ascontiguousarray(m_arr)

    xr = x.reshape(NCORES, BS, D)
    in_maps = []
    for c in range(NCORES):
        shard_t = np.ascontiguousarray(xr[c].T)  # [256, 4096] f32
        xh, xl = _split_bf16(shard_t)
        xin = np.stack([xh[:P], xh[P:], xl[:P], xl[P:]], axis=0)
        in_maps.append({"xin": np.ascontiguousarray(xin), "min": m_arr})
    return in_maps


def _gather(results) -> np.ndarray:
    out = np.empty((B, D), dtype=np.float32)
    for c in range(NCORES):
        outT = results[c]["outT"].reshape(D, BS)
        out[c * BS : (c + 1) * BS] = outT.T
    return out


def run(x: np.ndarray, theta: np.ndarray, trace: bool = False):
    """Returns (out, BassKernelResults)."""
    from concourse.bass_utils import run_bass_kernel_spmd

    in_maps = _make_in_maps(x, theta)
    res = run_bass_kernel_spmd(
        _get_nc(), in_maps, list(range(NCORES)), trace=trace
    )
    return _gather(res.results), res


def kernel(x: np.ndarray, theta: np.ndarray) -> np.ndarray:
    out, _ = run(x, theta, trace=False)
    return out
